# revision 1
# baseline (speedup 1.0000x reference)
"""Trainium2 Bass kernel for gated-adapter attention (Llama-Adapter style).

Sharding: 2 data-parallel groups of 4 cores (batch dim); within a group the 32
heads are tensor-parallel (8 heads/core).  Each core computes QKV + RoPE +
causal flash-style attention (transposed scores) + gated adapter cross
attention for its heads, AllGathers the per-head attention output across its
group of 4, then computes a column shard of the wo projection.  The host
reassembles the full [2, 2048, 4096] output from the 8 per-core shards.

All TensorEngine tensors are fp16 (1 cycle/row, ~0.05% rounding); PSUM
accumulation is fp32; softmax sums/reciprocals are fp32.
"""

import math
import os
import sys

import numpy as np

for _p in ("/opt/trn_rl_repo",):
    if os.path.isdir(_p) and _p not in sys.path:
        sys.path.insert(0, _p)

import ml_dtypes  # noqa: E402

import concourse.bass as bass  # noqa: E402
import concourse.mybir as mybir  # noqa: E402
import concourse.tile as tile  # noqa: E402
from concourse import bacc  # noqa: E402

FP16 = np.float16
F16 = mybir.dt.float16
F32 = mybir.dt.float32

DIM = 4096
S = 2048
B = 2
H = 32
HD = 128
ALEN = 10

NCORES = 8
CPG = 4          # cores per group (group = one batch element)
HPC = 8          # heads per core
OC = HPC * HD    # 1024 output dims per core for q/k/v and for wo columns

TCN = 17         # t-chunks of 128: 16 real + 1 adapter/pad chunk
TAUG = TCN * 128  # 2176
NM = DIM // 128   # 32 contraction chunks
QB = 4           # query blocks
QW = 512         # query block width
SCALE = 1.0 / math.sqrt(HD)

REPLICA_GROUPS = [[0, 1, 2, 3], [4, 5, 6, 7]]

Exp = mybir.ActivationFunctionType.Exp
Copy = mybir.ActivationFunctionType.Copy
MULT = None  # set after import in _alu()


def _alu():
    from concourse.alu_op_type import AluOpType
    return AluOpType


def build_graph():
    nc = bacc.Bacc(
        "TRN2",
        target_bir_lowering=False,
        debug=False,
        num_devices=NCORES,
    )

    # ---- external I/O ------------------------------------------------------
    xT = nc.dram_tensor("xT", [DIM, TAUG], F16, kind="ExternalInput")
    wqT = nc.dram_tensor("wqT", [DIM, OC], F16, kind="ExternalInput")
    wkT = nc.dram_tensor("wkT", [DIM, OC], F16, kind="ExternalInput")
    wvT = nc.dram_tensor("wvT", [DIM, OC], F16, kind="ExternalInput")
    woT = nc.dram_tensor("woT", [DIM, OC], F16, kind="ExternalInput")
    cosP = nc.dram_tensor("cosP", [S, 64], F16, kind="ExternalInput")
    sinP = nc.dram_tensor("sinP", [S, 64], F16, kind="ExternalInput")
    maskmul = nc.dram_tensor("maskmul", [QB, 4, 128, QW], F16, kind="ExternalInput")
    gates = nc.dram_tensor("gates", [16, HPC], F32, kind="ExternalInput")
    eye = nc.dram_tensor("eye", [128, 128], F16, kind="ExternalInput")
    out_ext = nc.dram_tensor("out", [S, OC], F32, kind="ExternalOutput")

    op = _alu()

    with tile.TileContext(nc) as tc:
        with (
            tc.tile_pool(name="persist", bufs=1) as persist,
            tc.tile_pool(name="dram", bufs=1, space="DRAM") as dpool,
        ):
            kT = persist.tile([128, HPC * S], F16, tag="kT")     # [d, h*t]
            vsb = persist.tile([128, TCN * OC], F16, tag="vsb")  # [t, tc*o]
            akT = persist.tile([128, HPC * 16], F16, tag="akT")  # [d, h*16]
            avg = persist.tile([16, HPC * HD], F16, tag="avg")   # [a, h*d]
            ones = persist.tile([128, 1], F16, tag="ones")
            eyesb = persist.tile([128, 128], F16, tag="eyesb")
            gatesb = persist.tile([16, HPC], F32, tag="gatesb")
            cossb = persist.tile([128, 16 * 64], F16, tag="cossb")
            sinsb = persist.tile([128, 16 * 64], F16, tag="sinsb")
            negC = persist.tile([128, 1], F32, tag="negC")

            nc.gpsimd.memset(ones[:], 1.0)
            nc.gpsimd.memset(negC[:], -9.0)
            nc.sync.dma_start(eyesb[:], eye[:])
            nc.sync.dma_start(gatesb[:], gates[:])
            nc.sync.dma_start(
                cossb[:].rearrange("p (c d) -> p c d", c=16),
                cosP[:].rearrange("(c p) d -> p c d", p=128),
            )
            nc.sync.dma_start(
                sinsb[:].rearrange("p (c d) -> p c d", c=16),
                sinP[:].rearrange("(c p) d -> p c d", p=128),
            )

            qT = persist.tile([128, HPC * S], F16, tag="qT")
            agin = [dpool.tile([OC, QW], F16, tag=f"agin{q}", name=f"agin{q}")
                    for q in range(QB)]
            agout = [dpool.tile([CPG * OC, QW], F16, tag=f"agout{q}",
                                name=f"agout{q}")
                     for q in range(QB)]

            # ================= Phase 1: QKV + RoPE + transposes ============
            with (
                tc.tile_pool(name="wres", bufs=36) as wpool,
                tc.tile_pool(name="xin", bufs=6) as xpool,
                tc.tile_pool(name="asm", bufs=3) as apool,
                tc.tile_pool(name="rot", bufs=3) as rpool,
                tc.tile_pool(name="ps1", bufs=2, space="PSUM") as pspool,
                tc.tile_pool(name="pstr", bufs=2, space="PSUM") as ptpool,
            ):
                for proj, wsrc in (("q", wqT), ("k", wkT), ("v", wvT)):
                    if proj == "q":
                        tch_list = list(range(16))
                    elif proj == "k":
                        tch_list = list(range(TCN))
                    else:
                        tch_list = [16] + list(range(16))
                    wres = []
                    for m in range(NM):
                        wt = wpool.tile([128, OC], F16, tag="wres",
                                        name=f"w{proj}{m}")
                        nc.sync.dma_start(
                            wt[:], wsrc[m * 128:(m + 1) * 128, :])
                        wres.append(wt)
                    for tch in tch_list:
                        ps = pspool.tile([128, OC], F32, tag="ps1",
                                         name=f"ps{proj}{tch}")
                        for m in range(NM):
                            xm = xpool.tile([128, 128], F16, tag="xin",
                                            name=f"x{proj}{tch}_{m}")
                            nc.sync.dma_start(
                                xm[:],
                                xT[m * 128:(m + 1) * 128,
                                   tch * 128:(tch + 1) * 128],
                            )
                            for half in range(2):
                                nc.tensor.matmul(
                                    ps[:, half * 512:(half + 1) * 512],
                                    lhsT=xm[:],
                                    rhs=wres[m][:, half * 512:
                                                 (half + 1) * 512],
                                    start=(m == 0),
                                    stop=(m == NM - 1),
                                )
                        if proj == "v":
                            nc.scalar.activation(
                                vsb[:, tch * OC:tch * OC + 512],
                                ps[:, 0:512], Copy)
                            nc.vector.tensor_copy(
                                vsb[:, tch * OC + 512:(tch + 1) * OC],
                                ps[:, 512:1024])
                            if tch == 16:
                                for h in range(HPC):
                                    nc.vector.tensor_scalar(
                                        avg[0:10, h * HD:(h + 1) * HD],
                                        vsb[0:10, 16 * OC + h * HD:
                                            16 * OC + (h + 1) * HD],
                                        gatesb[0:10, h:h + 1],
                                        None,
                                        op.mult,
                                    )
                            continue
                        asmt = apool.tile([128, OC], F16, tag="asm",
                                          name=f"as{proj}{tch}")
                        nc.scalar.activation(asmt[:, 0:512], ps[:, 0:512],
                                             Copy)
                        nc.vector.tensor_copy(asmt[:, 512:1024],
                                              ps[:, 512:1024])
                        if tch == 16:
                            # adapter chunk (k only): transpose into akT
                            ptr = ptpool.tile([128, OC], F16, tag="pstr",
                                              name="ptrak")
                            for h in range(HPC):
                                nc.tensor.transpose(
                                    ptr[:, h * 128:(h + 1) * 128],
                                    asmt[:, h * 128:(h + 1) * 128],
                                    eyesb[:],
                                )
                            nc.scalar.activation(
                                akT[:].rearrange("p (h a) -> p h a", h=HPC),
                                ptr[:].rearrange(
                                    "p (h t) -> p h t", h=HPC)[:, :, 0:16],
                                Copy,
                            )
                            continue
                        # ---- RoPE (deinterleaved head_dim layout) --------
                        a3 = asmt[:].rearrange(
                            "p (h d) -> p h d", h=HPC)[:, :, 0:64]
                        b3 = asmt[:].rearrange(
                            "p (h d) -> p h d", h=HPC)[:, :, 64:128]
                        cos1 = cossb[:].rearrange(
                            "p (c o d) -> p c o d", c=16, o=1)[:, tch]
                        sin1 = sinsb[:].rearrange(
                            "p (c o d) -> p c o d", c=16, o=1)[:, tch]
                        cos3, _ = bass.broadcast_tensor_aps(cos1, a3)
                        sin3, _ = bass.broadcast_tensor_aps(sin1, a3)
                        rot = rpool.tile([128, OC], F16, tag="rot",
                                         name=f"rot{proj}{tch}")
                        ra = rot[:].rearrange(
                            "p (h d) -> p h d", h=HPC)[:, :, 0:64]
                        rb = rot[:].rearrange(
                            "p (h d) -> p h d", h=HPC)[:, :, 64:128]
                        t1 = rpool.tile([128, HPC * 64], F16, tag="rt1",
                                        name=f"rt1{proj}{tch}")
                        t13 = t1[:].rearrange("p (h d) -> p h d", h=HPC)
                        t2 = rpool.tile([128, HPC * 64], F16, tag="rt2",
                                        name=f"rt2{proj}{tch}")
                        t23 = t2[:].rearrange("p (h d) -> p h d", h=HPC)
                        nc.vector.tensor_tensor(t13, a3, cos3, op.mult)
                        nc.vector.tensor_tensor(t23, b3, sin3, op.mult)
                        nc.vector.tensor_tensor(ra, t13, t23, op.subtract)
                        nc.vector.tensor_tensor(t13, a3, sin3, op.mult)
                        nc.vector.tensor_tensor(t23, b3, cos3, op.mult)
                        nc.vector.tensor_tensor(rb, t13, t23, op.add)
                        # ---- transpose per head -> kT sbuf / qT dram -----
                        ptr = ptpool.tile([128, OC], F16, tag="pstr",
                                          name=f"ptr{proj}{tch}")
                        for h in range(HPC):
                            nc.tensor.transpose(
                                ptr[:, h * 128:(h + 1) * 128],
                                rot[:, h * 128:(h + 1) * 128],
                                eyesb[:],
                            )
                        dst = kT if proj == "k" else qT
                        nc.scalar.activation(
                            dst[:].rearrange(
                                "p (h t) -> p h t",
                                h=HPC)[:, :, tch * 128:(tch + 1) * 128],
                            ptr[:].rearrange("p (h d) -> p h d", h=HPC),
                            Copy,
                        )

            # ============ Phase 2+3: attention / AllGather / wo ============
            from contextlib import ExitStack
            _es = ExitStack()
            with _es:
                P = lambda **kw: _es.enter_context(tc.tile_pool(**kw))
                prpool = P(name="probs", bufs=10)
                mkpool = P(name="mask", bufs=4)
                smpool = P(name="small", bufs=3)
                rcpool = P(name="rec", bufs=2)
                bcpool = P(name="bcast", bufs=2)
                ctpool = P(name="ctmp", bufs=2)
                pscp = P(name="psc", bufs=2, space="PSUM")
                ppvp = P(name="ppv", bufs=2, space="PSUM")
                psmp = P(name="psums", bufs=1, space="PSUM")
                psap = P(name="psumA", bufs=1, space="PSUM")
                def attention_block(qb):
                    kk = (qb + 1) * 4  # causal: k chunks 0..kk-1
                    mtiles = []
                    for dk in range(4):
                        mt = mkpool.tile([128, QW], F16, tag="mask",
                                         name=f"mm{qb}{dk}")
                        nc.sync.dma_start(mt[:], maskmul[qb, dk])
                        mtiles.append(mt)
                    for h in range(HPC):
                        q_ap = qT[:, h * S + qb * QW: h * S + (qb + 1) * QW]
                        sums = psmp.tile([1, QW], F32, tag="sums",
                                         name=f"sums{qb}_{h}")
                        sumA = psap.tile([1, QW], F32, tag="sumA",
                                         name=f"sumA{qb}_{h}")
                        asc = pscp.tile([10, QW], F32, tag="sc",
                                        name=f"asc{qb}_{h}")
                        pbs = []
                        for kc in range(kk):
                            sc = pscp.tile([128, QW], F32, tag="sc",
                                           name=f"sc{qb}_{h}_{kc}")
                            nc.tensor.matmul(
                                sc[:],
                                lhsT=kT[:, h * S + kc * 128:
                                        h * S + (kc + 1) * 128],
                                rhs=q_ap,
                                start=True, stop=True,
                            )
                            pb = prpool.tile([128, QW], F16, tag="probs",
                                             name=f"pb{qb}_{h}_{kc}")
                            nc.scalar.activation(pb[:], sc[:], Exp,
                                                 bias=negC[:, 0:1],
                                                 scale=SCALE)
                            if kc >= qb * 4:
                                nc.vector.tensor_tensor(
                                    pb[:], pb[:],
                                    mtiles[kc - qb * 4][:], op.mult)
                            pbs.append(pb)
                        pv = ppvp.tile([128, QW], F32, tag="pv",
                                       name=f"pv{qb}_{h}")
                        for i in range(kk):
                            nc.tensor.matmul(
                                sums[:], lhsT=ones[:, 0:1], rhs=pbs[i][:],
                                start=(i == 0), stop=(i == kk - 1),
                            )
                            nc.tensor.matmul(
                                pv[:],
                                lhsT=vsb[:, i * OC + h * HD:
                                         i * OC + (h + 1) * HD],
                                rhs=pbs[i][:],
                                start=(i == 0), stop=(i == kk - 1),
                            )
                        # adapter
                        nc.tensor.matmul(
                            asc[:], lhsT=akT[:, h * 16:h * 16 + 10],
                            rhs=q_ap, start=True, stop=True)
                        apb = smpool.tile([10, QW], F16, tag="aprobs",
                                          name=f"apb{qb}_{h}")
                        nc.scalar.activation(apb[:], asc[:], Exp,
                                             bias=negC[0:10, 0:1],
                                             scale=SCALE)
                        nc.tensor.matmul(
                            sumA[:], lhsT=ones[0:10, 0:1], rhs=apb[:],
                            start=True, stop=True)
                        apv = ppvp.tile([128, QW], F32, tag="pv",
                                        name=f"apv{qb}_{h}")
                        nc.tensor.matmul(
                            apv[:], lhsT=avg[0:10, h * HD:(h + 1) * HD],
                            rhs=apb[:], start=True, stop=True)
                        # normalize + combine
                        recM = rcpool.tile([1, QW], F32, tag="recM",
                                           name=f"rM{qb}_{h}")
                        nc.vector.reciprocal_approx_fast(recM[:], sums[:])
                        recA = rcpool.tile([1, QW], F32, tag="recA",
                                           name=f"rA{qb}_{h}")
                        nc.vector.reciprocal_approx_fast(recA[:], sumA[:])
                        bcM = bcpool.tile([128, QW], F32, tag="bcM",
                                          name=f"bM{qb}_{h}")
                        nc.gpsimd.partition_broadcast(bcM[:], recM[:])
                        bcA = bcpool.tile([128, QW], F32, tag="bcA",
                                          name=f"bA{qb}_{h}")
                        nc.gpsimd.partition_broadcast(bcA[:], recA[:])
                        c1 = ctpool.tile([128, QW], F32, tag="c1",
                                         name=f"c1{qb}_{h}")
                        nc.vector.tensor_tensor(c1[:], pv[:], bcM[:], op.mult)
                        c2 = ctpool.tile([128, QW], F32, tag="c2",
                                         name=f"c2{qb}_{h}")
                        nc.vector.tensor_tensor(c2[:], apv[:], bcA[:],
                                                op.mult)
                        c3 = ctpool.tile([128, QW], F16, tag="c3",
                                         name=f"c3{qb}_{h}")
                        nc.vector.tensor_tensor(c3[:], c1[:], c2[:], op.add)
                        nc.sync.dma_start(
                            agin[qb][h * 128:(h + 1) * 128, :], c3[:])
                    nc.gpsimd.collective_compute(
                        "AllGather",
                        op.bypass,
                        replica_groups=REPLICA_GROUPS,
                        ins=[agin[qb][:].opt()],
                        outs=[agout[qb][:].opt()],
                    )

                for qb in range(QB - 1, -1, -1):
                    attention_block(qb)

            # ================= Phase 3: wo projection ======================
            _es2 = ExitStack()
            with _es2:
                P2 = lambda **kw: _es2.enter_context(tc.tile_pool(**kw))
                w2pool = P2(name="w2", bufs=34)
                agpool = P2(name="agsb", bufs=36)
                ostpool = P2(name="ost", bufs=3)
                pwop = P2(name="pwo", bufs=2, space="PSUM")

                def load_w2(jh):
                    tiles = []
                    for m in range(NM):
                        wt = w2pool.tile([128, 512], F16, tag="w2",
                                         name=f"w2_{jh}_{m}")
                        nc.sync.dma_start(
                            wt[:], woT[m * 128:(m + 1) * 128,
                                       jh * 512:(jh + 1) * 512])
                        tiles.append(wt)
                    return tiles

                def wo_chunk(jh, w2t, qb):
                    ag = []
                    for i in range(NM):
                        a = agpool.tile([128, QW], F16, tag="agsb",
                                        name=f"ag{jh}_{qb}_{i}")
                        nc.sync.dma_start(
                            a[:], agout[qb][i * 128:(i + 1) * 128, :])
                        ag.append(a)
                    for tsub in range(4):
                        ps = pwop.tile([128, 512], F32, tag="pwo",
                                       name=f"pwo{jh}{qb}{tsub}")
                        for i in range(NM):
                            nc.tensor.matmul(
                                ps[:],
                                lhsT=ag[i][:, tsub * 128:(tsub + 1) * 128],
                                rhs=w2t[i][:],
                                start=(i == 0), stop=(i == NM - 1),
                            )
                        st = ostpool.tile([128, 512], F32, tag="ost",
                                          name=f"st{jh}{qb}{tsub}")
                        nc.scalar.activation(st[:], ps[:], Copy)
                        r0 = qb * QW + tsub * 128
                        nc.sync.dma_start(
                            out_ext[r0:r0 + 128,
                                    jh * 512:(jh + 1) * 512], st[:])

                for jh in range(2):
                    w2t = load_w2(jh)
                    for qb in range(QB - 1, -1, -1):
                        wo_chunk(jh, w2t, qb)

    nc.compile()
    return nc


# ---------------------------------------------------------------------------
# host-side input prep + execution
# ---------------------------------------------------------------------------

_DEINT = np.concatenate([np.arange(0, 128, 2), np.arange(1, 128, 2)])


def _prep_inputs(x, adapter, wq, wk, wv, wo, gate, freqs_cos, freqs_sin, mask):
    """Build the per-core input maps."""
    perm = np.concatenate([h * HD + _DEINT for h in range(H)])  # deinterleave
    wqp = wq[perm, :]  # permute output dims of wq/wk for rope layout
    wkp = wk[perm, :]

    in_maps = []
    for c in range(NCORES):
        g, ci = divmod(c, CPG)
        osl = slice(ci * OC, (ci + 1) * OC)
        xT = np.zeros((DIM, TAUG), FP16)
        xT[:, :S] = x[g].T.astype(FP16)
        xT[:, S:S + ALEN] = adapter[0].T.astype(FP16)
        mm = np.empty((QB, 4, 128, QW), FP16)
        for qb in range(QB):
            q0 = qb * QW
            for dk in range(4):
                k0 = q0 + dk * 128
                mm[qb, dk] = np.exp(
                    mask[0, 0, q0:q0 + QW, k0:k0 + 128]).T.astype(FP16)
        gates = np.zeros((16, HPC), np.float32)
        gates[:, :] = gate[0, ci * HPC:(ci + 1) * HPC, 0, 0][None, :]
        in_maps.append({
            "xT": xT,
            "wqT": np.ascontiguousarray(wqp[osl].T).astype(FP16),
            "wkT": np.ascontiguousarray(wkp[osl].T).astype(FP16),
            "wvT": np.ascontiguousarray(wv[osl].T).astype(FP16),
            "woT": np.ascontiguousarray(wo[osl].T).astype(FP16),
            "cosP": freqs_cos.astype(FP16),
            "sinP": freqs_sin.astype(FP16),
            "maskmul": mm,
            "gates": gates,
            "eye": np.eye(128, dtype=FP16),
        })
    return in_maps


_NC_CACHE = {}
TRACE = bool(int(os.environ.get("BASS_KERNEL_TRACE", "0")))
LAST_EXEC_NS = None
LAST_RESULTS = None


def kernel(x, adapter, wq, wk, wv, wo, gate, freqs_cos, freqs_sin, mask,
           start_pos=0, **_unused):
    global LAST_EXEC_NS, LAST_RESULTS
    from concourse.bass_utils import run_bass_kernel_spmd

    to_np = lambda a: np.asarray(a)
    x, adapter, wq, wk, wv, wo = map(to_np, (x, adapter, wq, wk, wv, wo))
    gate, freqs_cos, freqs_sin, mask = map(
        to_np, (gate, freqs_cos, freqs_sin, mask))

    if "nc" not in _NC_CACHE:
        _NC_CACHE["nc"] = build_graph()
    nc = _NC_CACHE["nc"]

    in_maps = _prep_inputs(x, adapter, wq, wk, wv, wo, gate,
                           freqs_cos, freqs_sin, mask)
    res = run_bass_kernel_spmd(
        nc, in_maps, core_ids=list(range(NCORES)), trace=TRACE)
    LAST_EXEC_NS = res.exec_time_ns
    LAST_RESULTS = res
    out = np.empty((B, S, DIM), np.float32)
    for c in range(NCORES):
        g, ci = divmod(c, CPG)
        out[g, :, ci * OC:(ci + 1) * OC] = res.results[c]["out"]
    return out



# revision 13
# speedup vs baseline: 1.3461x; 1.3461x over previous
"""Trainium2 Bass kernel for gated-adapter attention (Llama-Adapter style).

Sharding: 2 data-parallel groups of 4 cores (batch dim); within a group the 32
heads are tensor-parallel (8 heads/core).  Each core computes QKV + RoPE +
causal attention (transposed scores) + gated adapter cross attention for its
heads, AllGathers the per-head attention output across its group of 4, then
computes a column shard of the wo projection.  The host reassembles the full
[2, 2048, 4096] output from the 8 per-core shards.

v2 restructure (trace-driven):
 - softmax denominators via fp16 pair-tree on DVE + one ones-matmul per
   (head, qblock) instead of one ones-matmul per key chunk (PE -150us).
 - exp computed on [128,1024] chunk pairs (halves ACT overhead).
 - causal mask via a single 128x128 triangle tile + zero-fill (no mask DMAs).
 - projections run m-outer over [128,256] x tiles (bigger DMA lines, tiny
   x pool); weight DMAs interleaved with first x tiles so PE starts early.
 - q blocks kept in SBUF; wo weights prefetched under attention; wo follows
   immediately after the last attention block.
"""

import math
import os
import sys

import numpy as np

for _p in ("/opt/trn_rl_repo",):
    if os.path.isdir(_p) and _p not in sys.path:
        sys.path.insert(0, _p)

import ml_dtypes  # noqa: E402

import concourse.bass as bass  # noqa: E402
import concourse.mybir as mybir  # noqa: E402
import concourse.tile as tile  # noqa: E402
from concourse import bacc  # noqa: E402
from concourse import bass_isa  # noqa: E402

FP16 = np.float16
F16 = mybir.dt.float16
F32 = mybir.dt.float32

DIM = 4096
S = 2048
B = 2
H = 32
HD = 128
ALEN = 10

NCORES = 8
CPG = 4          # cores per group (group = one batch element)
HPC = 8          # heads per core
OC = HPC * HD    # 1024 output dims per core for q/k/v and for wo columns

TCN = 17         # t-chunks of 128: 16 real + 1 adapter/pad chunk
TAUG = TCN * 128  # 2176
NM = DIM // 128   # 32 contraction chunks
QB = 4           # query blocks
QW = 512         # query block width
SCALE = 1.0 / math.sqrt(HD)

REPLICA_GROUPS = [[0, 1, 2, 3], [4, 5, 6, 7]]

Exp = mybir.ActivationFunctionType.Exp
Copy = mybir.ActivationFunctionType.Copy


def _alu():
    from concourse.alu_op_type import AluOpType
    return AluOpType


def build_graph():
    nc = bacc.Bacc(
        "TRN2",
        target_bir_lowering=False,
        debug=False,
        num_devices=NCORES,
    )
    op = _alu()

    # ---- external I/O ------------------------------------------------------
    xT = nc.dram_tensor("xT", [DIM, TAUG], F16, kind="ExternalInput")
    wqT = nc.dram_tensor("wqT", [DIM, OC], F16, kind="ExternalInput")
    wkT = nc.dram_tensor("wkT", [DIM, OC], F16, kind="ExternalInput")
    wvT = nc.dram_tensor("wvT", [DIM, OC], F16, kind="ExternalInput")
    woT = nc.dram_tensor("woT", [DIM, OC], F16, kind="ExternalInput")
    cosS = nc.dram_tensor("cosS", [128, 16 * 64], F16, kind="ExternalInput")
    sinS = nc.dram_tensor("sinS", [128, 16 * 64], F16, kind="ExternalInput")
    tri = nc.dram_tensor("tri", [128, 128], F16, kind="ExternalInput")
    gates = nc.dram_tensor("gates", [16, HPC], F32, kind="ExternalInput")
    eye = nc.dram_tensor("eye", [128, 128], F16, kind="ExternalInput")
    out_ext = nc.dram_tensor("out", [S, OC], F32, kind="ExternalOutput")

    from contextlib import ExitStack
    with tile.TileContext(nc) as tc:
        with (
            tc.tile_pool(name="persist", bufs=1) as persist,
            tc.tile_pool(name="dram", bufs=1, space="DRAM") as dpool,
            tc.tile_pool(name="qst", bufs=4) as qpool,
        ):
            # ---- persistent tiles ------------------------------------------
            kT = persist.tile([128, HPC * S], F16, tag="kT")     # [d, h*t]
            vsb = persist.tile([128, 16 * OC], F16, tag="vsb")   # [t, tc*o]
            akT = persist.tile([128, HPC * 16], F16, tag="akT")  # [d, h*16]
            avg = persist.tile([16, HPC * HD], F16, tag="avg")   # [a, h*d]
            ones = persist.tile([128, 1], F16, tag="ones")
            eyesb = persist.tile([128, 128], F16, tag="eyesb")
            trisb = persist.tile([128, 128], F16, tag="trisb")
            gatesb = persist.tile([16, HPC], F32, tag="gatesb")
            cossb = persist.tile([128, 16 * 64], F16, tag="cossb")
            sinsb = persist.tile([128, 16 * 64], F16, tag="sinsb")
            negC = persist.tile([128, 1], F32, tag="negC")

            nc.gpsimd.memset(ones[:], 1.0)
            nc.gpsimd.memset(negC[:], -9.0)
            nc.sync.dma_start(eyesb[:], eye[:])
            nc.sync.dma_start(trisb[:], tri[:])
            nc.sync.dma_start(gatesb[:], gates[:])
            nc.sync.dma_start(cossb[:], cosS[:])
            nc.sync.dma_start(sinsb[:], sinS[:])

            qstage = {}
            agin = [dpool.tile([OC, QW], F16, tag=f"agin{q}", name=f"agin{q}")
                    for q in range(QB)]
            agout = [dpool.tile([CPG * OC, QW], F16, tag=f"agout{q}",
                                name=f"agout{q}")
                     for q in range(QB)]

            # =============== phase A: QKV projections =======================
            es_a = ExitStack()
            with es_a:
                PA = lambda **kw: es_a.enter_context(tc.tile_pool(**kw))
                wpool = PA(name="wres", bufs=36)
                xpool = PA(name="xin", bufs=8)
                apool = PA(name="asm", bufs=2)
                rpool = PA(name="rot", bufs=2)
                rtpool = PA(name="rt", bufs=6)
                pspool = PA(name="ps1", bufs=3, space="PSUM")
                ptpool = PA(name="pstr", bufs=2, space="PSUM")

                def rope_and_store(proj, tch, asmt):
                    """RoPE on asmt [t,d], transpose into kT/qstage."""
                    a3 = asmt[:].rearrange(
                        "p (h d) -> p h d", h=HPC)[:, :, 0:64]
                    b3 = asmt[:].rearrange(
                        "p (h d) -> p h d", h=HPC)[:, :, 64:128]
                    cos1 = cossb[:].rearrange(
                        "p (c o d) -> p c o d", c=16, o=1)[:, tch]
                    sin1 = sinsb[:].rearrange(
                        "p (c o d) -> p c o d", c=16, o=1)[:, tch]
                    cos3, _ = bass.broadcast_tensor_aps(cos1, a3)
                    sin3, _ = bass.broadcast_tensor_aps(sin1, a3)
                    rot = rpool.tile([128, OC], F16, tag="rot",
                                     name=f"rot{proj}{tch}")
                    ra = rot[:].rearrange(
                        "p (h d) -> p h d", h=HPC)[:, :, 0:64]
                    rb = rot[:].rearrange(
                        "p (h d) -> p h d", h=HPC)[:, :, 64:128]
                    t1 = rtpool.tile([128, HPC * 64], F16, tag="rt",
                                     name=f"rt1{proj}{tch}")
                    t13 = t1[:].rearrange("p (h d) -> p h d", h=HPC)
                    t2 = rtpool.tile([128, HPC * 64], F16, tag="rt",
                                     name=f"rt2{proj}{tch}")
                    t23 = t2[:].rearrange("p (h d) -> p h d", h=HPC)
                    nc.vector.tensor_tensor(t13, a3, cos3, op.mult)
                    nc.vector.tensor_tensor(t23, b3, sin3, op.mult)
                    nc.vector.tensor_tensor(ra, t13, t23, op.subtract)
                    nc.vector.tensor_tensor(t13, a3, sin3, op.mult)
                    nc.vector.tensor_tensor(t23, b3, cos3, op.mult)
                    nc.vector.tensor_tensor(rb, t13, t23, op.add)
                    ptr = ptpool.tile([128, OC], F16, tag="pstr",
                                      name=f"ptr{proj}{tch}")
                    for h in range(HPC):
                        nc.tensor.transpose(
                            ptr[:, h * 128:(h + 1) * 128],
                            rot[:, h * 128:(h + 1) * 128],
                            eyesb[:],
                        )
                    if proj == "k":
                        nc.scalar.activation(
                            kT[:].rearrange(
                                "p (h t) -> p h t",
                                h=HPC)[:, :, tch * 128:(tch + 1) * 128],
                            ptr[:].rearrange("p (h d) -> p h d", h=HPC),
                            Copy,
                        )
                    else:  # q
                        qb = tch // 4
                        off = (tch % 4) * 128
                        nc.scalar.activation(
                            qstage[qb][:].rearrange(
                                "p (h t) -> p h t",
                                h=HPC)[:, :, off:off + 128],
                            ptr[:].rearrange("p (h d) -> p h d", h=HPC),
                            Copy,
                        )

                def proj_chunk(proj, tch, ps):
                    """Post-matmul processing for one [128, OC] psum chunk."""
                    if proj == "v":
                        if tch == 16:  # adapter values -> gated avg
                            for h in range(HPC):
                                nc.vector.tensor_scalar(
                                    avg[0:10, h * HD:(h + 1) * HD],
                                    ps[0:10, h * HD:(h + 1) * HD],
                                    gatesb[0:10, h:h + 1],
                                    None,
                                    op.mult,
                                )
                            return
                        nc.scalar.activation(
                            vsb[:, tch * OC:tch * OC + 512],
                            ps[:, 0:512], Copy)
                        nc.vector.tensor_copy(
                            vsb[:, tch * OC + 512:(tch + 1) * OC],
                            ps[:, 512:1024])
                        return
                    asmt = apool.tile([128, OC], F16, tag="asm",
                                      name=f"as{proj}{tch}")
                    nc.scalar.activation(asmt[:, 0:512], ps[:, 0:512], Copy)
                    nc.vector.tensor_copy(asmt[:, 512:1024], ps[:, 512:1024])
                    if tch == 16:  # adapter chunk (k only): no rope
                        ptr = ptpool.tile([128, OC], F16, tag="pstr",
                                          name="ptrak")
                        for h in range(HPC):
                            nc.tensor.transpose(
                                ptr[:, h * 128:(h + 1) * 128],
                                asmt[:, h * 128:(h + 1) * 128],
                                eyesb[:],
                            )
                        nc.scalar.activation(
                            akT[:].rearrange("p (h a) -> p h a", h=HPC),
                            ptr[:].rearrange(
                                "p (h t) -> p h t", h=HPC)[:, :, 0:16],
                            Copy,
                        )
                        return
                    rope_and_store(proj, tch, asmt)

                def proj_group(proj, wres, chs, first_group=False, wsrc=None):
                    """Group of chunks, m-outer: x tile used 2x then freed."""
                    W = len(chs) * 128
                    c0 = chs[0] * 128
                    psl = [pspool.tile([128, OC], F32, tag="ps1",
                                       name=f"ps{proj}{tch}")
                           for tch in chs]
                    for m in range(NM):
                        if first_group and wsrc is not None:
                            nc.sync.dma_start(
                                wres[m][:], wsrc[m * 128:(m + 1) * 128, :])
                        xt = xpool.tile([128, W], F16, tag="xin",
                                        name=f"x{proj}{chs[0]}_{m}")
                        nc.sync.dma_start(
                            xt[:], xT[m * 128:(m + 1) * 128, c0:c0 + W])
                        for j in range(len(chs)):
                            for half in range(2):
                                nc.tensor.matmul(
                                    psl[j][:, half * 512:(half + 1) * 512],
                                    lhsT=xt[:, j * 128:(j + 1) * 128],
                                    rhs=wres[m][:,
                                                half * 512:(half + 1) * 512],
                                    start=(m == 0),
                                    stop=(m == NM - 1),
                                )
                    for j, tch in enumerate(chs):
                        proj_chunk(proj, tch, psl[j])

                def make_wres(proj):
                    return [wpool.tile([128, OC], F16, tag="wres",
                                       name=f"w{proj}{m}")
                            for m in range(NM)]

                def run_proj(proj, wsrc, chunks):
                    wres = make_wres(proj)
                    groups = [chunks[i:i + 2]
                              for i in range(0, len(chunks), 2)]
                    for gi, chs in enumerate(groups):
                        proj_group(proj, wres, chs,
                                   first_group=(gi == 0), wsrc=wsrc)

                run_proj("k", wkT, list(range(16)) + [16])
                run_proj("v", wvT, list(range(16)) + [16])
                for qb in range(QB):
                    qstage[qb] = qpool.tile([128, HPC * QW], F16,
                                            tag="qstage", name=f"qs{qb}")
                run_proj("q", wqT, list(range(16)))

            # =============== phase B: attention =============================
            es_w = ExitStack()
            es_b = ExitStack()
            with es_w, es_b:
                w2pool = es_w.enter_context(tc.tile_pool(name="w2", bufs=64))
                PB = lambda **kw: es_b.enter_context(tc.tile_pool(**kw))
                prpool = PB(name="probs", bufs=3)
                partpool = PB(name="part", bufs=10)
                appool = PB(name="aprobs", bufs=2)
                recpool = PB(name="rec", bufs=1)
                bcpool = PB(name="bcast", bufs=1)
                ctpool = PB(name="ctmp", bufs=4)
                copool = PB(name="cout", bufs=2)
                pscp = PB(name="psc", bufs=2, space="PSUM")
                ppvp = PB(name="ppv", bufs=2, space="PSUM")
                psmp = PB(name="psm", bufs=1, space="PSUM")
                pascp = PB(name="pas", bufs=1, space="PSUM")

                def attention_block(qb):
                    kk = (qb + 1) * 4
                    npairs = kk // 2
                    qs = qstage[qb]
                    for h in range(HPC):
                        q_ap = qs[:, h * QW:(h + 1) * QW]
                        # adapter scores early (overlap with main loop)
                        asc = pascp.tile([10, QW], F32, tag="pas",
                                         name=f"asc{qb}_{h}")
                        nc.tensor.matmul(
                            asc[:], lhsT=akT[:, h * 16:h * 16 + 10],
                            rhs=q_ap, start=True, stop=True)
                        apb = appool.tile([10, QW], F16, tag="aprobs",
                                          name=f"apb{qb}_{h}")
                        nc.scalar.activation(apb[:], asc[:], Exp,
                                             bias=negC[0:10, 0:1],
                                             scale=SCALE)
                        sA = appool.tile([10, QW], F32, tag="sA",
                                         name=f"sA{qb}_{h}")
                        nc.gpsimd.partition_all_reduce(
                            sA[:], apb[:], 10, bass_isa.ReduceOp.add)
                        # main causal attention in chunk pairs
                        pv = ppvp.tile([128, QW], F32, tag="pv",
                                       name=f"pv{qb}_{h}")
                        partials = []
                        for pr in range(npairs):
                            sc = pscp.tile([128, 2 * QW], F32, tag="sc",
                                           name=f"sc{qb}_{h}_{pr}")
                            for half in range(2):
                                kc = 2 * pr + half
                                nc.tensor.matmul(
                                    sc[:, half * QW:(half + 1) * QW],
                                    lhsT=kT[:, h * S + kc * 128:
                                            h * S + (kc + 1) * 128],
                                    rhs=q_ap,
                                    start=True, stop=True,
                                )
                            pb = prpool.tile([128, 2 * QW], F16, tag="probs",
                                             name=f"pb{qb}_{h}_{pr}")
                            nc.scalar.activation(pb[:], sc[:], Exp,
                                                 bias=negC[:, 0:1],
                                                 scale=SCALE)
                            if pr >= qb * 2:  # diagonal pair: causal mask
                                for half in range(2):
                                    dk = 2 * pr + half - qb * 4
                                    o = half * QW
                                    if dk > 0:
                                        nc.vector.tensor_scalar(
                                            pb[:, o:o + dk * 128],
                                            pb[:, o:o + dk * 128],
                                            0.0, None, op.mult)
                                    nc.vector.tensor_tensor(
                                        pb[:, o + dk * 128:
                                           o + (dk + 1) * 128],
                                        pb[:, o + dk * 128:
                                           o + (dk + 1) * 128],
                                        trisb[:], op.mult)
                            part = partpool.tile([128, QW], F16, tag="part",
                                                 name=f"pp{qb}_{h}_{pr}")
                            nc.vector.tensor_tensor(part[:], pb[:, 0:QW],
                                                    pb[:, QW:2 * QW], op.add)
                            partials.append(part)
                            for half in range(2):
                                kc = 2 * pr + half
                                nc.tensor.matmul(
                                    pv[:],
                                    lhsT=vsb[:, kc * OC + h * HD:
                                             kc * OC + (h + 1) * HD],
                                    rhs=pb[:, half * QW:(half + 1) * QW],
                                    start=(kc == 0), stop=(kc == kk - 1),
                                )
                        # adapter values
                        apv = ppvp.tile([128, QW], F32, tag="pv",
                                        name=f"apv{qb}_{h}")
                        nc.tensor.matmul(
                            apv[:], lhsT=avg[0:10, h * HD:(h + 1) * HD],
                            rhs=apb[:], start=True, stop=True)
                        # fp16 pairwise tree -> per-column sums
                        while len(partials) > 1:
                            nxt = []
                            for i in range(0, len(partials) - 1, 2):
                                t = partpool.tile(
                                    [128, QW], F16, tag="part",
                                    name=f"tr{qb}_{h}_"
                                         f"{len(partials)}_{i}")
                                nc.vector.tensor_tensor(
                                    t[:], partials[i][:],
                                    partials[i + 1][:], op.add)
                                nxt.append(t)
                            if len(partials) % 2:
                                nxt.append(partials[-1])
                            partials = nxt
                        sums = psmp.tile([1, QW], F32, tag="psm",
                                         name=f"sums{qb}_{h}")
                        nc.tensor.matmul(
                            sums[:], lhsT=ones[:, 0:1],
                            rhs=partials[0][:], start=True, stop=True)
                        # reciprocals + broadcast
                        recMA = recpool.tile([1, 2 * QW], F32, tag="rec",
                                             name=f"rec{qb}_{h}")
                        nc.vector.reciprocal_approx_fast(
                            recMA[0:1, 0:QW], sums[:])
                        nc.vector.reciprocal_approx_fast(
                            recMA[0:1, QW:2 * QW], sA[0:1, :])
                        bcMA = bcpool.tile([128, 2 * QW], F32, tag="bcast",
                                           name=f"bc{qb}_{h}")
                        nc.gpsimd.partition_broadcast(bcMA[:], recMA[:])
                        c1 = ctpool.tile([128, QW], F16, tag="ctmp",
                                         name=f"c1{qb}_{h}")
                        nc.vector.tensor_tensor(c1[:], pv[:], bcMA[:, 0:QW],
                                                op.mult)
                        c2 = ctpool.tile([128, QW], F16, tag="ctmp",
                                         name=f"c2{qb}_{h}")
                        nc.vector.tensor_tensor(c2[:], apv[:],
                                                bcMA[:, QW:2 * QW], op.mult)
                        c3 = copool.tile([128, QW], F16, tag="cout",
                                         name=f"c3{qb}_{h}")
                        nc.vector.tensor_tensor(c3[:], c1[:], c2[:], op.add)
                        nc.sync.dma_start(
                            agin[qb][h * 128:(h + 1) * 128, :], c3[:])
                    nc.gpsimd.collective_compute(
                        "AllGather",
                        op.bypass,
                        replica_groups=REPLICA_GROUPS,
                        ins=[agin[qb][:].opt()],
                        outs=[agout[qb][:].opt()],
                    )

                w2t = {0: [], 1: []}
                attention_block(0)
                # wo weight prefetch hides under remaining attention
                for jh in range(2):
                    for m in range(NM):
                        wt = w2pool.tile([128, 512], F16, tag="w2",
                                         name=f"w2_{jh}_{m}")
                        nc.sync.dma_start(
                            wt[:], woT[m * 128:(m + 1) * 128,
                                       jh * 512:(jh + 1) * 512])
                        w2t[jh].append(wt)
                for qb in range(1, QB):
                    attention_block(qb)
                es_b.close()

                # =============== phase C: wo projection =====================
                es_c = ExitStack()
                with es_c:
                    PC = lambda **kw: es_c.enter_context(tc.tile_pool(**kw))
                    agpool = PC(name="agsb", bufs=34)
                    ostpool = PC(name="ost", bufs=2)
                    pwop = PC(name="pwo", bufs=2, space="PSUM")

                    for jh in range(2):
                        for qb in range(QB):
                            ag = []
                            for i in range(NM):
                                a = agpool.tile([128, QW], F16, tag="agsb",
                                                name=f"ag{jh}_{qb}_{i}")
                                nc.sync.dma_start(
                                    a[:],
                                    agout[qb][i * 128:(i + 1) * 128, :])
                                ag.append(a)
                            for tsub in range(4):
                                ps = pwop.tile([128, 512], F32, tag="pwo",
                                               name=f"pwo{jh}{qb}{tsub}")
                                for i in range(NM):
                                    nc.tensor.matmul(
                                        ps[:],
                                        lhsT=ag[i][:, tsub * 128:
                                                   (tsub + 1) * 128],
                                        rhs=w2t[jh][i][:],
                                        start=(i == 0), stop=(i == NM - 1),
                                    )
                                st = ostpool.tile([128, 512], F32, tag="ost",
                                                  name=f"st{jh}{qb}{tsub}")
                                nc.scalar.activation(st[:], ps[:], Copy)
                                r0 = qb * QW + tsub * 128
                                nc.sync.dma_start(
                                    out_ext[r0:r0 + 128,
                                            jh * 512:(jh + 1) * 512], st[:])

    nc.compile()
    return nc


# ---------------------------------------------------------------------------
# host-side input prep + execution
# ---------------------------------------------------------------------------

_DEINT = np.concatenate([np.arange(0, 128, 2), np.arange(1, 128, 2)])


def _prep_inputs(x, adapter, wq, wk, wv, wo, gate, freqs_cos, freqs_sin, mask):
    """Build the per-core input maps."""
    perm = np.concatenate([h * HD + _DEINT for h in range(H)])  # deinterleave
    wqp = wq[perm, :]  # permute output dims of wq/wk for rope layout
    wkp = wk[perm, :]

    # cos/sin tables pre-laid for SBUF: [p, c*64] with p = t within chunk
    cosS = np.ascontiguousarray(
        freqs_cos.reshape(16, 128, 64).transpose(1, 0, 2).reshape(128, 1024)
    ).astype(FP16)
    sinS = np.ascontiguousarray(
        freqs_sin.reshape(16, 128, 64).transpose(1, 0, 2).reshape(128, 1024)
    ).astype(FP16)
    # 128x128 causal triangle (transposed): tri[k, q] = exp(mask)[q, k]
    tri = np.ascontiguousarray(
        np.exp(mask[0, 0, 0:128, 0:128]).T).astype(FP16)

    in_maps = []
    for c in range(NCORES):
        g, ci = divmod(c, CPG)
        osl = slice(ci * OC, (ci + 1) * OC)
        xTh = np.zeros((DIM, TAUG), FP16)
        xTh[:, :S] = x[g].T.astype(FP16)
        xTh[:, S:S + ALEN] = adapter[0].T.astype(FP16)
        gatesh = np.zeros((16, HPC), np.float32)
        gatesh[:, :] = gate[0, ci * HPC:(ci + 1) * HPC, 0, 0][None, :]
        in_maps.append({
            "xT": xTh,
            "wqT": np.ascontiguousarray(wqp[osl].T).astype(FP16),
            "wkT": np.ascontiguousarray(wkp[osl].T).astype(FP16),
            "wvT": np.ascontiguousarray(wv[osl].T).astype(FP16),
            "woT": np.ascontiguousarray(wo[osl].T).astype(FP16),
            "cosS": cosS,
            "sinS": sinS,
            "tri": tri,
            "gates": gatesh,
            "eye": np.eye(128, dtype=FP16),
        })
    return in_maps


_NC_CACHE = {}
TRACE = bool(int(os.environ.get("BASS_KERNEL_TRACE", "0")))
LAST_EXEC_NS = None
LAST_RESULTS = None


def kernel(x, adapter, wq, wk, wv, wo, gate, freqs_cos, freqs_sin, mask,
           start_pos=0, **_unused):
    global LAST_EXEC_NS, LAST_RESULTS
    from concourse.bass_utils import run_bass_kernel_spmd

    to_np = lambda a: np.asarray(a)
    x, adapter, wq, wk, wv, wo = map(to_np, (x, adapter, wq, wk, wv, wo))
    gate, freqs_cos, freqs_sin, mask = map(
        to_np, (gate, freqs_cos, freqs_sin, mask))

    if "nc" not in _NC_CACHE:
        _NC_CACHE["nc"] = build_graph()
    nc = _NC_CACHE["nc"]

    in_maps = _prep_inputs(x, adapter, wq, wk, wv, wo, gate,
                           freqs_cos, freqs_sin, mask)
    res = run_bass_kernel_spmd(
        nc, in_maps, core_ids=list(range(NCORES)), trace=TRACE)
    LAST_EXEC_NS = res.exec_time_ns
    LAST_RESULTS = res
    out = np.empty((B, S, DIM), np.float32)
    for c in range(NCORES):
        g, ci = divmod(c, CPG)
        out[g, :, ci * OC:(ci + 1) * OC] = res.results[c]["out"]
    return out


# revision 14
# speedup vs baseline: 1.3777x; 1.0235x over previous
"""Trainium2 Bass kernel for gated-adapter attention (Llama-Adapter style).

Sharding: 2 data-parallel groups of 4 cores (batch dim); within a group the 32
heads are tensor-parallel (8 heads/core).  Each core computes QKV + RoPE +
causal attention (transposed scores) + gated adapter cross attention for its
heads, AllGathers the per-head attention output across its group of 4, then
computes a column shard of the wo projection.  The host reassembles the full
[2, 2048, 4096] output from the 8 per-core shards.

v2 restructure (trace-driven):
 - softmax denominators via fp16 pair-tree on DVE + one ones-matmul per
   (head, qblock) instead of one ones-matmul per key chunk (PE -150us).
 - exp computed on [128,1024] chunk pairs (halves ACT overhead).
 - causal mask via a single 128x128 triangle tile + zero-fill (no mask DMAs).
 - projections run m-outer over [128,256] x tiles (bigger DMA lines, tiny
   x pool); weight DMAs interleaved with first x tiles so PE starts early.
 - q blocks kept in SBUF; wo weights prefetched under attention; wo follows
   immediately after the last attention block.
"""

import math
import os
import sys

import numpy as np

for _p in ("/opt/trn_rl_repo",):
    if os.path.isdir(_p) and _p not in sys.path:
        sys.path.insert(0, _p)

import ml_dtypes  # noqa: E402

import concourse.bass as bass  # noqa: E402
import concourse.mybir as mybir  # noqa: E402
import concourse.tile as tile  # noqa: E402
from concourse import bacc  # noqa: E402
from concourse import bass_isa  # noqa: E402

FP16 = np.float16
F16 = mybir.dt.float16
F32 = mybir.dt.float32

DIM = 4096
S = 2048
B = 2
H = 32
HD = 128
ALEN = 10

NCORES = 8
CPG = 4          # cores per group (group = one batch element)
HPC = 8          # heads per core
OC = HPC * HD    # 1024 output dims per core for q/k/v and for wo columns

TCN = 17         # t-chunks of 128: 16 real + 1 adapter/pad chunk
TAUG = TCN * 128  # 2176
NM = DIM // 128   # 32 contraction chunks
QB = 4           # query blocks
QW = 512         # query block width
SCALE = 1.0 / math.sqrt(HD)

REPLICA_GROUPS = [[0, 1, 2, 3], [4, 5, 6, 7]]

Exp = mybir.ActivationFunctionType.Exp
Copy = mybir.ActivationFunctionType.Copy


def _alu():
    from concourse.alu_op_type import AluOpType
    return AluOpType


def build_graph():
    nc = bacc.Bacc(
        "TRN2",
        target_bir_lowering=False,
        debug=False,
        num_devices=NCORES,
    )
    op = _alu()

    # ---- external I/O ------------------------------------------------------
    xT = nc.dram_tensor("xT", [DIM, TAUG], F16, kind="ExternalInput")
    wqT = nc.dram_tensor("wqT", [DIM, OC], F16, kind="ExternalInput")
    wkT = nc.dram_tensor("wkT", [DIM, OC], F16, kind="ExternalInput")
    wvT = nc.dram_tensor("wvT", [DIM, OC], F16, kind="ExternalInput")
    woT = nc.dram_tensor("woT", [DIM, OC], F16, kind="ExternalInput")
    cosS = nc.dram_tensor("cosS", [128, 16 * 64], F16, kind="ExternalInput")
    sinS = nc.dram_tensor("sinS", [128, 16 * 64], F16, kind="ExternalInput")
    tri = nc.dram_tensor("tri", [128, 128], F16, kind="ExternalInput")
    gates = nc.dram_tensor("gates", [16, HPC], F32, kind="ExternalInput")
    eye = nc.dram_tensor("eye", [128, 128], F16, kind="ExternalInput")
    out_ext = nc.dram_tensor("out", [S, OC], F32, kind="ExternalOutput")

    from contextlib import ExitStack
    with tile.TileContext(nc) as tc:
        with (
            tc.tile_pool(name="persist", bufs=1) as persist,
            tc.tile_pool(name="dram", bufs=1, space="DRAM") as dpool,
            tc.tile_pool(name="qst", bufs=4) as qpool,
        ):
            # ---- persistent tiles ------------------------------------------
            kT = persist.tile([128, HPC * S], F16, tag="kT")     # [d, h*t]
            vsb = persist.tile([128, 16 * OC], F16, tag="vsb")   # [t, tc*o]
            akT = persist.tile([128, HPC * 16], F16, tag="akT")  # [d, h*16]
            avg = persist.tile([16, HPC * HD], F16, tag="avg")   # [a, h*d]
            ones = persist.tile([128, 1], F16, tag="ones")
            eyesb = persist.tile([128, 128], F16, tag="eyesb")
            trisb = persist.tile([128, 128], F16, tag="trisb")
            gatesb = persist.tile([16, HPC], F32, tag="gatesb")
            cossb = persist.tile([128, 16 * 64], F16, tag="cossb")
            sinsb = persist.tile([128, 16 * 64], F16, tag="sinsb")
            negC = persist.tile([128, 1], F32, tag="negC")

            nc.gpsimd.memset(ones[:], 1.0)
            nc.gpsimd.memset(negC[:], -9.0)
            nc.sync.dma_start(eyesb[:], eye[:])
            nc.sync.dma_start(trisb[:], tri[:])
            nc.sync.dma_start(gatesb[:], gates[:])
            nc.sync.dma_start(cossb[:], cosS[:])
            nc.sync.dma_start(sinsb[:], sinS[:])

            qstage = {}
            agin = [dpool.tile([OC, QW], F16, tag=f"agin{q}", name=f"agin{q}")
                    for q in range(QB)]
            agout = [dpool.tile([CPG * OC, QW], F16, tag=f"agout{q}",
                                name=f"agout{q}")
                     for q in range(QB)]

            # =============== phase A: QKV projections =======================
            es_a = ExitStack()
            with es_a:
                PA = lambda **kw: es_a.enter_context(tc.tile_pool(**kw))
                wpool = PA(name="wres", bufs=36)
                xpool = PA(name="xin", bufs=8)
                apool = PA(name="asm", bufs=2)
                rpool = PA(name="rot", bufs=2)
                rtpool = PA(name="rt", bufs=6)
                pspool = PA(name="ps1", bufs=3, space="PSUM")
                ptpool = PA(name="pstr", bufs=2, space="PSUM")

                def rope_and_store(proj, tch, asmt):
                    """RoPE on asmt [t,d], transpose into kT/qstage."""
                    a3 = asmt[:].rearrange(
                        "p (h d) -> p h d", h=HPC)[:, :, 0:64]
                    b3 = asmt[:].rearrange(
                        "p (h d) -> p h d", h=HPC)[:, :, 64:128]
                    cos1 = cossb[:].rearrange(
                        "p (c o d) -> p c o d", c=16, o=1)[:, tch]
                    sin1 = sinsb[:].rearrange(
                        "p (c o d) -> p c o d", c=16, o=1)[:, tch]
                    cos3, _ = bass.broadcast_tensor_aps(cos1, a3)
                    sin3, _ = bass.broadcast_tensor_aps(sin1, a3)
                    rot = rpool.tile([128, OC], F16, tag="rot",
                                     name=f"rot{proj}{tch}")
                    ra = rot[:].rearrange(
                        "p (h d) -> p h d", h=HPC)[:, :, 0:64]
                    rb = rot[:].rearrange(
                        "p (h d) -> p h d", h=HPC)[:, :, 64:128]
                    t1 = rtpool.tile([128, HPC * 64], F16, tag="rt",
                                     name=f"rt1{proj}{tch}")
                    t13 = t1[:].rearrange("p (h d) -> p h d", h=HPC)
                    t2 = rtpool.tile([128, HPC * 64], F16, tag="rt",
                                     name=f"rt2{proj}{tch}")
                    t23 = t2[:].rearrange("p (h d) -> p h d", h=HPC)
                    nc.vector.tensor_tensor(t13, a3, cos3, op.mult)
                    nc.vector.tensor_tensor(t23, b3, sin3, op.mult)
                    nc.vector.tensor_tensor(ra, t13, t23, op.subtract)
                    nc.vector.tensor_tensor(t13, a3, sin3, op.mult)
                    nc.vector.tensor_tensor(t23, b3, cos3, op.mult)
                    nc.vector.tensor_tensor(rb, t13, t23, op.add)
                    ptr = ptpool.tile([128, OC], F16, tag="pstr",
                                      name=f"ptr{proj}{tch}")
                    for h in range(HPC):
                        nc.tensor.transpose(
                            ptr[:, h * 128:(h + 1) * 128],
                            rot[:, h * 128:(h + 1) * 128],
                            eyesb[:],
                        )
                    if proj == "k":
                        nc.scalar.activation(
                            kT[:].rearrange(
                                "p (h t) -> p h t",
                                h=HPC)[:, :, tch * 128:(tch + 1) * 128],
                            ptr[:].rearrange("p (h d) -> p h d", h=HPC),
                            Copy,
                        )
                    else:  # q
                        qb = tch // 4
                        off = (tch % 4) * 128
                        nc.scalar.activation(
                            qstage[qb][:].rearrange(
                                "p (h t) -> p h t",
                                h=HPC)[:, :, off:off + 128],
                            ptr[:].rearrange("p (h d) -> p h d", h=HPC),
                            Copy,
                        )

                def proj_chunk(proj, tch, ps):
                    """Post-matmul processing for one [128, OC] psum chunk."""
                    if proj == "v":
                        if tch == 16:  # adapter values -> gated avg
                            for h in range(HPC):
                                nc.vector.tensor_scalar(
                                    avg[0:10, h * HD:(h + 1) * HD],
                                    ps[0:10, h * HD:(h + 1) * HD],
                                    gatesb[0:10, h:h + 1],
                                    None,
                                    op.mult,
                                )
                            return
                        nc.scalar.activation(
                            vsb[:, tch * OC:tch * OC + 512],
                            ps[:, 0:512], Copy)
                        nc.vector.tensor_copy(
                            vsb[:, tch * OC + 512:(tch + 1) * OC],
                            ps[:, 512:1024])
                        return
                    asmt = apool.tile([128, OC], F16, tag="asm",
                                      name=f"as{proj}{tch}")
                    nc.scalar.activation(asmt[:, 0:512], ps[:, 0:512], Copy)
                    nc.vector.tensor_copy(asmt[:, 512:1024], ps[:, 512:1024])
                    if tch == 16:  # adapter chunk (k only): no rope
                        ptr = ptpool.tile([128, OC], F16, tag="pstr",
                                          name="ptrak")
                        for h in range(HPC):
                            nc.tensor.transpose(
                                ptr[:, h * 128:(h + 1) * 128],
                                asmt[:, h * 128:(h + 1) * 128],
                                eyesb[:],
                            )
                        nc.scalar.activation(
                            akT[:].rearrange("p (h a) -> p h a", h=HPC),
                            ptr[:].rearrange(
                                "p (h t) -> p h t", h=HPC)[:, :, 0:16],
                            Copy,
                        )
                        return
                    rope_and_store(proj, tch, asmt)

                def proj_group(proj, wres, chs, first_group=False, wsrc=None):
                    """Group of chunks, m-outer: x tile used 2x then freed."""
                    W = len(chs) * 128
                    c0 = chs[0] * 128
                    psl = [pspool.tile([128, OC], F32, tag="ps1",
                                       name=f"ps{proj}{tch}")
                           for tch in chs]
                    for m in range(NM):
                        if first_group and wsrc is not None:
                            nc.sync.dma_start(
                                wres[m][:], wsrc[m * 128:(m + 1) * 128, :])
                        xt = xpool.tile([128, W], F16, tag="xin",
                                        name=f"x{proj}{chs[0]}_{m}")
                        nc.sync.dma_start(
                            xt[:], xT[m * 128:(m + 1) * 128, c0:c0 + W])
                        for j in range(len(chs)):
                            for half in range(2):
                                nc.tensor.matmul(
                                    psl[j][:, half * 512:(half + 1) * 512],
                                    lhsT=xt[:, j * 128:(j + 1) * 128],
                                    rhs=wres[m][:,
                                                half * 512:(half + 1) * 512],
                                    start=(m == 0),
                                    stop=(m == NM - 1),
                                )
                    for j, tch in enumerate(chs):
                        proj_chunk(proj, tch, psl[j])

                def make_wres(proj):
                    return [wpool.tile([128, OC], F16, tag="wres",
                                       name=f"w{proj}{m}")
                            for m in range(NM)]

                def run_proj(proj, wsrc, chunks):
                    wres = make_wres(proj)
                    groups = [chunks[i:i + 2]
                              for i in range(0, len(chunks), 2)]
                    for gi, chs in enumerate(groups):
                        proj_group(proj, wres, chs,
                                   first_group=(gi == 0), wsrc=wsrc)

                run_proj("k", wkT, list(range(16)) + [16])
                run_proj("v", wvT, list(range(16)) + [16])
                for qb in range(QB):
                    qstage[qb] = qpool.tile([128, HPC * QW], F16,
                                            tag="qstage", name=f"qs{qb}")
                run_proj("q", wqT, list(range(16)))

            # =============== phase B: attention =============================
            es_w = ExitStack()
            es_b = ExitStack()
            with es_w, es_b:
                w2pool = es_w.enter_context(tc.tile_pool(name="w2", bufs=64))
                PB = lambda **kw: es_b.enter_context(tc.tile_pool(**kw))
                prpool = PB(name="probs", bufs=3)
                partpool = PB(name="part", bufs=10)
                appool = PB(name="aprobs", bufs=2)
                recpool = PB(name="rec", bufs=1)
                bcpool = PB(name="bcast", bufs=1)
                ctpool = PB(name="ctmp", bufs=4)
                copool = PB(name="cout", bufs=2)
                pscp = PB(name="psc", bufs=2, space="PSUM")
                ppvp = PB(name="ppv", bufs=3, space="PSUM")
                psmp = PB(name="psm", bufs=1, space="PSUM")

                def attention_block(qb, pending):
                    kk = (qb + 1) * 4
                    npairs = kk // 2
                    qs = qstage[qb]

                    def head_tail(ctx):
                        # deferred softmax-denominator tail for a head
                        sums = psmp.tile([1, QW], F32, tag="psm",
                                         name="sums{}_{}".format(*ctx["id"]))
                        nc.tensor.matmul(
                            sums[:], lhsT=ones[:, 0:1],
                            rhs=ctx["acc"][:], start=True, stop=True)
                        recMA = recpool.tile([1, 2 * QW], F32, tag="rec",
                                             name="rec{}_{}".format(
                                                 *ctx["id"]))
                        nc.vector.reciprocal_approx_fast(
                            recMA[0:1, 0:QW], sums[:])
                        nc.vector.reciprocal_approx_fast(
                            recMA[0:1, QW:2 * QW], ctx["sA"][0:1, :])
                        bcMA = bcpool.tile([128, 2 * QW], F32, tag="bcast",
                                           name="bc{}_{}".format(*ctx["id"]))
                        nc.gpsimd.partition_broadcast(bcMA[:], recMA[:])
                        cqb, ch = ctx["id"]
                        c1 = ctpool.tile([128, QW], F16, tag="ctmp",
                                         name=f"c1{cqb}_{ch}")
                        nc.vector.tensor_tensor(c1[:], ctx["pv"][:],
                                                bcMA[:, 0:QW], op.mult)
                        c2 = ctpool.tile([128, QW], F16, tag="ctmp",
                                         name=f"c2{cqb}_{ch}")
                        nc.vector.tensor_tensor(c2[:], ctx["apv"][:],
                                                bcMA[:, QW:2 * QW], op.mult)
                        c3 = copool.tile([128, QW], F16, tag="cout",
                                         name=f"c3{cqb}_{ch}")
                        nc.vector.tensor_tensor(c3[:], c1[:], c2[:], op.add)
                        nc.sync.dma_start(
                            agin[cqb][ch * 128:(ch + 1) * 128, :], c3[:])

                    for h in range(HPC):
                        q_ap = qs[:, h * QW:(h + 1) * QW]
                        # adapter scores early (overlap with main loop)
                        asc = pscp.tile([10, QW], F32, tag="sc",
                                        name=f"asc{qb}_{h}")
                        nc.tensor.matmul(
                            asc[:], lhsT=akT[:, h * 16:h * 16 + 10],
                            rhs=q_ap, start=True, stop=True)
                        apb = appool.tile([10, QW], F16, tag="aprobs",
                                          name=f"apb{qb}_{h}")
                        nc.scalar.activation(apb[:], asc[:], Exp,
                                             bias=negC[0:10, 0:1],
                                             scale=SCALE)
                        sA = appool.tile([10, QW], F32, tag="sA",
                                         name=f"sA{qb}_{h}")
                        nc.gpsimd.partition_all_reduce(
                            sA[:], apb[:], 10, bass_isa.ReduceOp.add)
                        # main causal attention in chunk pairs
                        pv = ppvp.tile([128, QW], F32, tag="pv",
                                       name=f"pv{qb}_{h}")
                        acc = None
                        for pr in range(npairs):
                            sc = pscp.tile([128, 2 * QW], F32, tag="sc",
                                           name=f"sc{qb}_{h}_{pr}")
                            for half in range(2):
                                kc = 2 * pr + half
                                nc.tensor.matmul(
                                    sc[:, half * QW:(half + 1) * QW],
                                    lhsT=kT[:, h * S + kc * 128:
                                            h * S + (kc + 1) * 128],
                                    rhs=q_ap,
                                    start=True, stop=True,
                                )
                            if pr == 0 and pending[0] is not None:
                                # previous head's tail: its sums matmul
                                # slots between our score and pv matmuls
                                head_tail(pending[0])
                                pending[0] = None
                            pb = prpool.tile([128, 2 * QW], F16, tag="probs",
                                             name=f"pb{qb}_{h}_{pr}")
                            nc.scalar.activation(pb[:], sc[:], Exp,
                                                 bias=negC[:, 0:1],
                                                 scale=SCALE)
                            if pr >= qb * 2:  # diagonal pair: causal mask
                                for half in range(2):
                                    dk = 2 * pr + half - qb * 4
                                    o = half * QW
                                    if dk > 0:
                                        nc.vector.tensor_scalar(
                                            pb[:, o:o + dk * 128],
                                            pb[:, o:o + dk * 128],
                                            0.0, None, op.mult)
                                    nc.vector.tensor_tensor(
                                        pb[:, o + dk * 128:
                                           o + (dk + 1) * 128],
                                        pb[:, o + dk * 128:
                                           o + (dk + 1) * 128],
                                        trisb[:], op.mult)
                            part = partpool.tile([128, QW], F16, tag="part",
                                                 name=f"pp{qb}_{h}_{pr}")
                            nc.vector.tensor_tensor(part[:], pb[:, 0:QW],
                                                    pb[:, QW:2 * QW], op.add)
                            if acc is None:
                                acc = part
                            else:
                                nacc = partpool.tile(
                                    [128, QW], F16, tag="part",
                                    name=f"acc{qb}_{h}_{pr}")
                                nc.vector.tensor_tensor(
                                    nacc[:], acc[:], part[:], op.add)
                                acc = nacc
                            for half in range(2):
                                kc = 2 * pr + half
                                nc.tensor.matmul(
                                    pv[:],
                                    lhsT=vsb[:, kc * OC + h * HD:
                                             kc * OC + (h + 1) * HD],
                                    rhs=pb[:, half * QW:(half + 1) * QW],
                                    start=(kc == 0), stop=(kc == kk - 1),
                                )
                        # adapter values
                        apv = ppvp.tile([128, QW], F32, tag="pv",
                                        name=f"apv{qb}_{h}")
                        nc.tensor.matmul(
                            apv[:], lhsT=avg[0:10, h * HD:(h + 1) * HD],
                            rhs=apb[:], start=True, stop=True)
                        pending[0] = {"id": (qb, h), "acc": acc, "sA": sA,
                                      "pv": pv, "apv": apv}
                    # flush the last head before the collective
                    head_tail(pending[0])
                    pending[0] = None
                    nc.gpsimd.collective_compute(
                        "AllGather",
                        op.bypass,
                        replica_groups=REPLICA_GROUPS,
                        ins=[agin[qb][:].opt()],
                        outs=[agout[qb][:].opt()],
                    )

                w2t = {0: [], 1: []}
                pending = [None]
                attention_block(0, pending)
                # wo weight prefetch hides under remaining attention
                for jh in range(2):
                    for m in range(NM):
                        wt = w2pool.tile([128, 512], F16, tag="w2",
                                         name=f"w2_{jh}_{m}")
                        nc.sync.dma_start(
                            wt[:], woT[m * 128:(m + 1) * 128,
                                       jh * 512:(jh + 1) * 512])
                        w2t[jh].append(wt)
                for qb in range(1, QB):
                    attention_block(qb, pending)
                es_b.close()

                # =============== phase C: wo projection =====================
                es_c = ExitStack()
                with es_c:
                    PC = lambda **kw: es_c.enter_context(tc.tile_pool(**kw))
                    agpool = PC(name="agsb", bufs=34)
                    ostpool = PC(name="ost", bufs=2)
                    pwop = PC(name="pwo", bufs=2, space="PSUM")

                    for qb in range(QB):
                        ag = []
                        for i in range(NM):
                            a = agpool.tile([128, QW], F16, tag="agsb",
                                            name=f"ag{qb}_{i}")
                            nc.sync.dma_start(
                                a[:],
                                agout[qb][i * 128:(i + 1) * 128, :])
                            ag.append(a)
                        for jh in range(2):
                            for tsub in range(4):
                                ps = pwop.tile([128, 512], F32, tag="pwo",
                                               name=f"pwo{jh}{qb}{tsub}")
                                for i in range(NM):
                                    nc.tensor.matmul(
                                        ps[:],
                                        lhsT=ag[i][:, tsub * 128:
                                                   (tsub + 1) * 128],
                                        rhs=w2t[jh][i][:],
                                        start=(i == 0), stop=(i == NM - 1),
                                    )
                                st = ostpool.tile([128, 512], F32, tag="ost",
                                                  name=f"st{jh}{qb}{tsub}")
                                nc.scalar.activation(st[:], ps[:], Copy)
                                r0 = qb * QW + tsub * 128
                                nc.sync.dma_start(
                                    out_ext[r0:r0 + 128,
                                            jh * 512:(jh + 1) * 512], st[:])

    nc.compile()
    return nc


# ---------------------------------------------------------------------------
# host-side input prep + execution
# ---------------------------------------------------------------------------

_DEINT = np.concatenate([np.arange(0, 128, 2), np.arange(1, 128, 2)])


def _prep_inputs(x, adapter, wq, wk, wv, wo, gate, freqs_cos, freqs_sin, mask):
    """Build the per-core input maps."""
    perm = np.concatenate([h * HD + _DEINT for h in range(H)])  # deinterleave
    wqp = wq[perm, :]  # permute output dims of wq/wk for rope layout
    wkp = wk[perm, :]

    # cos/sin tables pre-laid for SBUF: [p, c*64] with p = t within chunk
    cosS = np.ascontiguousarray(
        freqs_cos.reshape(16, 128, 64).transpose(1, 0, 2).reshape(128, 1024)
    ).astype(FP16)
    sinS = np.ascontiguousarray(
        freqs_sin.reshape(16, 128, 64).transpose(1, 0, 2).reshape(128, 1024)
    ).astype(FP16)
    # 128x128 causal triangle (transposed): tri[k, q] = exp(mask)[q, k]
    tri = np.ascontiguousarray(
        np.exp(mask[0, 0, 0:128, 0:128]).T).astype(FP16)

    in_maps = []
    for c in range(NCORES):
        g, ci = divmod(c, CPG)
        osl = slice(ci * OC, (ci + 1) * OC)
        xTh = np.zeros((DIM, TAUG), FP16)
        xTh[:, :S] = x[g].T.astype(FP16)
        xTh[:, S:S + ALEN] = adapter[0].T.astype(FP16)
        gatesh = np.zeros((16, HPC), np.float32)
        gatesh[:, :] = gate[0, ci * HPC:(ci + 1) * HPC, 0, 0][None, :]
        in_maps.append({
            "xT": xTh,
            "wqT": np.ascontiguousarray(wqp[osl].T).astype(FP16),
            "wkT": np.ascontiguousarray(wkp[osl].T).astype(FP16),
            "wvT": np.ascontiguousarray(wv[osl].T).astype(FP16),
            "woT": np.ascontiguousarray(wo[osl].T).astype(FP16),
            "cosS": cosS,
            "sinS": sinS,
            "tri": tri,
            "gates": gatesh,
            "eye": np.eye(128, dtype=FP16),
        })
    return in_maps


_NC_CACHE = {}
TRACE = bool(int(os.environ.get("BASS_KERNEL_TRACE", "0")))
LAST_EXEC_NS = None
LAST_RESULTS = None


def kernel(x, adapter, wq, wk, wv, wo, gate, freqs_cos, freqs_sin, mask,
           start_pos=0, **_unused):
    global LAST_EXEC_NS, LAST_RESULTS
    from concourse.bass_utils import run_bass_kernel_spmd

    to_np = lambda a: np.asarray(a)
    x, adapter, wq, wk, wv, wo = map(to_np, (x, adapter, wq, wk, wv, wo))
    gate, freqs_cos, freqs_sin, mask = map(
        to_np, (gate, freqs_cos, freqs_sin, mask))

    if "nc" not in _NC_CACHE:
        _NC_CACHE["nc"] = build_graph()
    nc = _NC_CACHE["nc"]

    in_maps = _prep_inputs(x, adapter, wq, wk, wv, wo, gate,
                           freqs_cos, freqs_sin, mask)
    res = run_bass_kernel_spmd(
        nc, in_maps, core_ids=list(range(NCORES)), trace=TRACE)
    LAST_EXEC_NS = res.exec_time_ns
    LAST_RESULTS = res
    out = np.empty((B, S, DIM), np.float32)
    for c in range(NCORES):
        g, ci = divmod(c, CPG)
        out[g, :, ci * OC:(ci + 1) * OC] = res.results[c]["out"]
    return out
